# revision 36
# baseline (speedup 1.0000x reference)
"""Multi-head attention (B=4, S=2048, D=1024, H=16, DH=64) on 8 TRN2 cores.

Sharding: core c -> (batch b = c//2, head-group g = c%2 of 8 heads).
Each core computes its batch's attention for its 8 heads plus the partial
W_O projection; the host sums the two partial outputs per batch (the
"all-reduce after W_O" done at unshard time).

Device kernel (per core), all matmul operands bf16 (inputs pre-cast on host):
  - QT/KT packed per head pair (128, S) bf16; V natural (128 sk, 8*65) bf16
    with a ones column per head (denominator via the attnV matmul).
  - scores^T per (head, q2, kc): psum (128 sk, 1024 sq) <- 2 matmuls;
    ScalarE exp (scale=0.125) -> bf16 P^T tiles (ring of 32).
  - attnV FLIPPED: po (128 sq, 65) = sum_kc P^T-chunk^T @ V_aug
    (lhsT = P^T (128 sk, 128 sq) stationary, rhs = V (128 sk, 65) moving):
    half the moving rows of the natural orientation.
  - normalize per-partition: recip(po[:,64]) -> tensor_scalar_mul -> bf16
    stage (128 sq, 128 dh-pair); dma_start_transpose -> otpair (128 dh2, S)
    (DMA xbar, off the compute engines).
  - out-proj: pf (128 sq, 512) accumulating 4 pair matmuls vs wo rows.

Schedule: ACT (exp, ~266us busy) is co-critical with PE (~274us busy).
Blocks run PAIR-MAJOR (h0q0, h0q1, h1q0, ... pairs adjacent) so projection
deadlines spread; remaining projections / V / out-proj chains are emitted
as cycle-budgeted "filler" between score tiles (ACT needs 1038ns/tile vs
PE's 427ns -> ~610ns/tile of filler budget); attnV chains of block i are
interleaved into block i+1's score loop.
"""

import sys

if "/opt/trn_rl_repo" not in sys.path:
    sys.path.insert(0, "/opt/trn_rl_repo")

import numpy as np
import ml_dtypes

import concourse.bass as bass
import concourse.tile as tile
from concourse import bacc
from concourse import mybir
from concourse import bass_utils

B, S, D, H, DH = 4, 2048, 1024, 16, 64
HL = 8              # heads per core
NCORES = 8
F32 = mybir.dt.float32
BF16 = mybir.dt.bfloat16
EXP = mybir.ActivationFunctionType.Exp

NDC = D // 128      # 8 d-chunks of 128
NKC = S // 128      # 16 s_k chunks of 128
NSL = 4             # x column slices of 512

DEBUG_DUMPS = False  # adds intermediate DRAM outputs for numeric bisection


def _kernel_body(tc):
    nc = tc.nc
    xT = nc.dram_tensor("xT", (D, S), BF16, kind="ExternalInput").ap()
    wq = nc.dram_tensor("wq", (D, HL * DH), BF16, kind="ExternalInput").ap()
    wk = nc.dram_tensor("wk", (D, HL * DH), BF16, kind="ExternalInput").ap()
    wv = nc.dram_tensor("wv", (D, HL * DH), BF16, kind="ExternalInput").ap()
    wo = nc.dram_tensor("wo", (HL * DH, D), BF16, kind="ExternalInput").ap()
    out = nc.dram_tensor("out", (S, D), F32, kind="ExternalOutput").ap()

    # load queues: NEVER the scalar queue -- HWDGE DMA triggers hold the
    # issuing engine's SEQ ~625ns each, and ACT's SEQ must stay free for exp
    qs = [nc.sync, nc.gpsimd]

    with tc.tile_pool(name="persist", bufs=1) as persist, \
         tc.tile_pool(name="ptp", bufs=32) as ptp, \
         tc.tile_pool(name="stgp", bufs=20) as stgp, \
         tc.tile_pool(name="rrp", bufs=4) as rrp, \
         tc.tile_pool(name="stp", bufs=5) as stp, \
         tc.tile_pool(name="psS", bufs=2, space="PSUM") as psS, \
         tc.tile_pool(name="psO", bufs=2, space="PSUM") as psO, \
         tc.tile_pool(name="psG", bufs=2, space="PSUM") as psG:

        # ---------------- persistent tiles ----------------
        xt = [persist.tile([128, S], BF16, name=f"xt{dc}", tag=f"xt{dc}")
              for dc in range(NDC)]
        wqt = [persist.tile([128, HL * DH], BF16, name=f"wq{dc}", tag=f"wq{dc}")
               for dc in range(NDC)]
        wkt = [persist.tile([128, HL * DH], BF16, name=f"wk{dc}", tag=f"wk{dc}")
               for dc in range(NDC)]
        wvt = [persist.tile([128, HL * DH], BF16, name=f"wv{dc}", tag=f"wv{dc}")
               for dc in range(NDC)]
        wo_t = [persist.tile([128, D], BF16, name=f"wo{p}", tag=f"wo{p}")
                for p in range(4)]
        qt = [persist.tile([128, S], BF16, name=f"qt{p}", tag=f"qt{p}") for p in range(4)]
        kt = [persist.tile([128, S], BF16, name=f"kt{p}", tag=f"kt{p}") for p in range(4)]
        vv = [persist.tile([128, HL * 65], BF16, name=f"v{sc}", tag=f"v{sc}")
              for sc in range(NKC)]
        otpair = {q2: [persist.tile([128, 1024], BF16, name=f"otp_{q2}_{p}",
                                    tag=f"otp_{q2}_{p}") for p in range(4)]
                  for q2 in range(2)}

        # -------- input DMAs: xc0, wq, xc1, wk, xc2, xc3, wv, wo ---------
        qi = [0]

        # 5:3 sync/gpsimd weighting -- sync's HWDGE trigger (625ns) outpaces
        # gpsimd's software-DGE (~1038ns on Pool), so balance completion
        LOADPAT = [0, 1, 0, 0, 1, 0, 0, 1]

        def load(dst, src):
            qs[LOADPAT[qi[0] % 8]].dma_start(out=dst, in_=src)
            qi[0] += 1

        def load_w(dst_list, src):
            for dc in range(NDC):
                load(dst_list[dc], src[dc * 128:(dc + 1) * 128, :])

        def load_xcol(sl, width=512):
            for dc in range(NDC):
                load(xt[dc][:, sl * 512:sl * 512 + width],
                     xT[dc * 128:(dc + 1) * 128, sl * 512:sl * 512 + width])

        # x cols 0:1024 as small early DMAs (prologue + early-score gates);
        # cols 1024:2048 as one batched DMA per d-chunk (fewer triggers)
        load_xcol(0)
        load_w(wqt, wq)
        load_w(wkt, wk)
        load_xcol(1)
        load_xcol(2, width=1024)
        load_w(wvt, wv)
        for p in range(4):
            load(wo_t[p], wo[p * 128:(p + 1) * 128, :])
        # explicit zero bias for exp: the implicit const AP would be DMA'd
        # behind all input loads on the shared DMA device, stalling exp #0
        zbias = persist.tile([128, 1], F32, name="zbias", tag="zbias")
        nc.vector.memset(zbias, 0.0)
        for sc in range(NKC):
            nc.vector.memset(vv[sc], 1.0)



        # ---------------- PE work-item generators (yield = step cycles) ---
        def proj_chain(kind, p, sl):
            """qt/kt chain: 8 accumulating matmuls -> copy to qt/kt bf16.
            Finish ops bundled with the LAST matmul (before final yield)."""
            w = wqt if kind == "q" else wkt
            dstl = qt if kind == "q" else kt
            ps = psG.tile([128, 512], F32, name=f"pg_{kind}{p}_{sl}", tag="pg")
            for dc in range(NDC):
                nc.tensor.matmul(ps, w[dc][:, p * 128:(p + 1) * 128],
                                 xt[dc][:, sl * 512:(sl + 1) * 512],
                                 start=(dc == 0), stop=(dc == NDC - 1))
                if dc == NDC - 1:
                    nc.vector.tensor_copy(dstl[p][:, sl * 512:(sl + 1) * 512], ps)
                yield 512

        def v_chain(hg, nh, sc):
            """V chains for head-group hg of width nh heads, sk-chunk sc.
            psum (128, nh*64) accumulated over dc, scattered into vv[sc]."""
            w0 = hg * 64
            ps = psG.tile([128, 512], F32, name=f"pgv_{hg}_{sc}", tag="pg")
            pv = ps[:, 0:nh * 64]
            for dc in range(NDC):
                nc.tensor.matmul(pv, xt[dc][:, sc * 128:(sc + 1) * 128],
                                 wvt[dc][:, w0:w0 + nh * 64],
                                 start=(dc == 0), stop=(dc == NDC - 1))
                if dc == NDC - 1:
                    vsrc = pv.rearrange("p (h x) -> p h x", x=64)
                    vdst = vv[sc].rearrange("p (h x) -> p h x", x=65)
                    nc.vector.tensor_copy(vdst[:, hg:hg + nh, 0:64], vsrc)
                if dc % 2 == 1:
                    yield nh * 128

        def outproj_chain(q2, j, dcol):
            """pf (128 sq, 512) = sum_p otpair[p][:, j]^T @ wo_t[p][:, dcol].
            q2=1 chains run in the tail where the scores ring is idle: give
            every other chain a psS slot so 4 PSUM bufs rotate, hiding the
            staging-copy turnaround."""
            if q2 == 1 and (j + dcol) % 2 == 0:
                pf = psS.tile([128, 1024], F32, name=f"pf_{q2}_{j}_{dcol}",
                              tag="ps")[:, 0:512]
            else:
                pf = psG.tile([128, 512], F32, name=f"pf_{q2}_{j}_{dcol}", tag="pg")
            for p in range(4):
                nc.tensor.matmul(pf, otpair[q2][p][:, j * 128:(j + 1) * 128],
                                 wo_t[p][:, dcol * 512:(dcol + 1) * 512],
                                 start=(p == 0), stop=(p == 3))
                if p == 3:
                    st = stp.tile([128, 512], F32, name=f"st_{q2}_{j}_{dcol}", tag="st")
                    if q2 == 1:
                        # tail: exps are done, ACT is idle -- keep DVE free
                        nc.scalar.copy(st, pf)
                    else:
                        nc.vector.tensor_copy(st, pf)
                    # sync queue: gpsimd's software-DGE trigger costs ~1us of
                    # Pool time per store and serializes the out-proj tail
                    nc.sync.dma_start(
                        out=out[q2 * 1024 + j * 128: q2 * 1024 + (j + 1) * 128,
                                dcol * 512:(dcol + 1) * 512],
                        in_=st)
                yield 512

        filler_q = []
        op_last = []    # q2 values whose j=7 out-proj is pending release

        def filler(budget):
            while op_last:
                q2l = op_last.pop(0)
                for dcol in range(2):
                    filler_q.append(outproj_chain(q2l, 7, dcol))
            while budget > 0 and filler_q:
                try:
                    budget -= next(filler_q[0])
                except StopIteration:
                    filler_q.pop(0)

        # ---------------- attention block machinery ----------------
        pts_map = {}
        stage_map = {}

        def attn_chain(blk, j):
            """Flipped attnV for sq-tile j of block blk, plus normalize,
            transpose, and (for the completing pair) out-proj scheduling."""
            h, q2 = blk
            p, rh = h // 2, h % 2
            pts = pts_map[blk]
            po = psO.tile([128, 65], F32, name=f"po_{h}_{q2}_{j}", tag="po")
            for kc in range(NKC):
                nc.tensor.matmul(po, pts[kc][:, j * 128:(j + 1) * 128],
                                 vv[kc][:, h * 65:(h + 1) * 65],
                                 start=(kc == 0), stop=(kc == NKC - 1))
            rr = rrp.tile([128, 1], F32, name=f"rr_{h}_{q2}_{j}", tag="rr")
            nc.vector.reciprocal(rr, po[:, 64:65])
            if rh == 0:
                stg = stgp.tile([128, 128], BF16, name=f"sg_{q2}_{p}_{j}", tag="sg")
                stage_map[(q2, p, j)] = stg
            else:
                stg = stage_map[(q2, p, j)]
            nc.vector.tensor_scalar_mul(stg[:, rh * 64:(rh + 1) * 64],
                                        po[:, 0:64], rr)
            if rh == 1:
                nc.sync.dma_start_transpose(
                    otpair[q2][p][:, j * 128:(j + 1) * 128], stg)
                if p == 3:
                    # release the out-proj chains of the PREVIOUS j now (its
                    # transpose DMA has settled), never the just-emitted one
                    # (head-of-line stall on the PE stream)
                    if j > 0:
                        for dcol in range(2):
                            filler_q.append(outproj_chain(q2, j - 1, dcol))
                    if j == 7:
                        op_last.append(q2)

        blk_prev = [None]

        def scores_block(blk, prev_start_kc=1, frate=1024):
            """Score+exp tiles for blk, interleaving prev block's attnV
            chains (j = (kc - prev_start_kc)//2 at odd offsets) + filler."""
            h, q2 = blk
            p, rh = h // 2, h % 2
            rsl = slice(rh * 64, (rh + 1) * 64)
            prev = blk_prev[0]
            pts = []
            pts_map[blk] = pts
            jdone = 0
            for kc in range(NKC):
                ps = psS.tile([128, 1024], F32, name=f"ps_{h}_{q2}_{kc}", tag="ps")
                for half in range(2):
                    nc.tensor.matmul(
                        ps[:, half * 512:(half + 1) * 512],
                        kt[p][rsl, kc * 128:(kc + 1) * 128],
                        qt[p][rsl, q2 * 1024 + half * 512: q2 * 1024 + (half + 1) * 512],
                        start=True, stop=True)
                pe = ptp.tile([128, 1024], BF16, name=f"pt_{h}_{q2}_{kc}", tag="pt")
                nc.scalar.activation(pe, ps, EXP, bias=zbias, scale=0.125)
                pts.append(pe)
                if (prev is not None and kc >= prev_start_kc
                        and (kc - prev_start_kc) % 2 == 0 and jdone < 8):
                    attn_chain(prev, jdone)
                    jdone += 1
                filler(frate)
            if prev is not None:
                while jdone < 8:
                    attn_chain(prev, jdone)
                    jdone += 1
                    filler(frate)
            blk_prev[0] = blk

        # ---------------- prologue: pair-0 qt/kt for first scores --------
        # direct: q-sl0, k-sl0, q-sl1 (kc0 scores need qt sl0+sl1, kt sl0);
        # k-sl1..3 + q-sl2,3 at the filler head (consumed before their kc).
        for chain in (proj_chain("q", 0, 0), proj_chain("k", 0, 0),
                      proj_chain("q", 0, 1)):
            for _ in chain:
                pass

        # filler supply in deadline order (see block order below):
        for sl in (1, 2, 3):
            filler_q.append(proj_chain("k", 0, sl))
        filler_q.append(proj_chain("q", 0, 2))
        filler_q.append(proj_chain("q", 0, 3))
        for sc in range(NKC):
            filler_q.append(v_chain(0, 1, sc))      # head 0 V (early deadline)
        for sc in range(NKC):
            filler_q.append(v_chain(1, 1, sc))      # head 1 V
        for p in range(1, 4):
            for sl in range(NSL):
                filler_q.append(proj_chain("k", p, sl))
            for sl in range(NSL):
                filler_q.append(proj_chain("q", p, sl))
            for sc in range(NKC):
                filler_q.append(v_chain(2 * p, 2, sc))  # pair p V

        # ---- main loop: pairs 0-2 pair-major (spreads projection/V filler
        # deadlines); pair 3 q2-major so attnV((7,0))'s transposes release
        # out-proj q2=0 two blocks before the end.
        blocks = [(0, 0), (0, 1), (1, 0), (1, 1), (2, 0), (2, 1), (3, 0), (3, 1),
                  (4, 0), (4, 1), (5, 0), (5, 1), (6, 0), (7, 0), (6, 1), (7, 1)]
        for i, blk in enumerate(blocks):
            # block0: no prev attnV -> spend the slack on filler.
            # block1: defer block0's attnV until head-0 V filler completes.
            scores_block(blk, prev_start_kc=7 if i == 1 else 1,
                         frate=1536 if i < 2 else 1024)

        # tail: attnV of the last block (all chains first -- their pair-3
        # transposes append the final out-proj chains), then drain.
        last = blk_prev[0]
        for j in range(8):
            attn_chain(last, j)
        while filler_q:
            filler(65536)

        if DEBUG_DUMPS:
            dbg = {
                "d_qt0": qt[0], "d_kt0": kt[0], "d_vv0": vv[0],
                "d_ot00": otpair[0][0], "d_ot03": otpair[0][3],
            }
            for nm, t_ in dbg.items():
                shp = (t_.shape[0], t_.shape[1])
                dt_ = nc.dram_tensor(nm, shp, t_.dtype, kind="ExternalOutput").ap()
                nc.sync.dma_start(out=dt_, in_=t_[:, :])


_NC_CACHE = None


def _get_nc():
    global _NC_CACHE
    if _NC_CACHE is None:
        nc = bacc.Bacc("TRN2", target_bir_lowering=False, debug=False)
        with tile.TileContext(nc) as tc:
            _kernel_body(tc)
        nc.compile()
        _NC_CACHE = nc
    return _NC_CACHE


def _shard_inputs(x, Wq, Wk, Wv, Wo):
    bf = ml_dtypes.bfloat16
    in_maps = []
    for c in range(NCORES):
        b, g = c // 2, c % 2
        xT = np.ascontiguousarray(x[b].T).astype(bf)
        sl = slice(HL * g, HL * (g + 1))
        wq_s = np.ascontiguousarray(Wq[sl].transpose(1, 0, 2).reshape(D, HL * DH)).astype(bf)
        wk_s = np.ascontiguousarray(Wk[sl].transpose(1, 0, 2).reshape(D, HL * DH)).astype(bf)
        wv_s = np.ascontiguousarray(Wv[sl].transpose(1, 0, 2).reshape(D, HL * DH)).astype(bf)
        wo_s = np.ascontiguousarray(Wo[HL * DH * g: HL * DH * (g + 1), :]).astype(bf)
        in_maps.append({"xT": xT, "wq": wq_s, "wk": wk_s, "wv": wv_s, "wo": wo_s})
    return in_maps


def kernel(**inputs):
    x = np.asarray(inputs["x"], dtype=np.float32)
    Wq = np.asarray(inputs["Wq"], dtype=np.float32)
    Wk = np.asarray(inputs["Wk"], dtype=np.float32)
    Wv = np.asarray(inputs["Wv"], dtype=np.float32)
    Wo = np.asarray(inputs["Wo"], dtype=np.float32)

    nc = _get_nc()
    in_maps = _shard_inputs(x, Wq, Wk, Wv, Wo)
    res = None
    for attempt in range(3):
        try:
            res = bass_utils.run_bass_kernel_spmd(nc, in_maps, core_ids=list(range(NCORES)))
            break
        except Exception:
            # transient axon/NRT device errors recover on retry
            if attempt == 2:
                raise
            import time
            time.sleep(20)
    outs = [res.results[c]["out"] for c in range(NCORES)]
    full = np.stack([outs[2 * b] + outs[2 * b + 1] for b in range(B)], axis=0)
    return full.astype(np.float32)


# revision 41
# speedup vs baseline: 1.0016x; 1.0016x over previous
"""Multi-head attention (B=4, S=2048, D=1024, H=16, DH=64) on 8 TRN2 cores.

Sharding: core c -> (batch b = c//2, head-group g = c%2 of 8 heads).
Each core computes its batch's attention for its 8 heads plus the partial
W_O projection; the host sums the two partial outputs per batch (the
"all-reduce after W_O" done at unshard time).

Device kernel (per core), all matmul operands bf16 (inputs pre-cast on host):
  - QT/KT packed per head pair (128, S) bf16; V natural (128 sk, 8*65) bf16
    with a ones column per head (denominator via the attnV matmul).
  - scores^T per (head, q2, kc): psum (128 sk, 1024 sq) <- 2 matmuls;
    ScalarE exp (scale=0.125) -> bf16 P^T tiles (ring of 32).
  - attnV FLIPPED: po (128 sq, 65) = sum_kc P^T-chunk^T @ V_aug
    (lhsT = P^T (128 sk, 128 sq) stationary, rhs = V (128 sk, 65) moving):
    half the moving rows of the natural orientation.
  - normalize per-partition: recip(po[:,64]) -> tensor_scalar_mul -> bf16
    stage (128 sq, 128 dh-pair); dma_start_transpose -> otpair (128 dh2, S)
    (DMA xbar, off the compute engines).
  - out-proj: pf (128 sq, 512) accumulating 4 pair matmuls vs wo rows.

Schedule: ACT (exp, ~266us busy) is co-critical with PE (~274us busy).
Blocks run PAIR-MAJOR (h0q0, h0q1, h1q0, ... pairs adjacent) so projection
deadlines spread; remaining projections / V / out-proj chains are emitted
as cycle-budgeted "filler" between score tiles (ACT needs 1038ns/tile vs
PE's 427ns -> ~610ns/tile of filler budget); attnV chains of block i are
interleaved into block i+1's score loop.
"""

import sys

if "/opt/trn_rl_repo" not in sys.path:
    sys.path.insert(0, "/opt/trn_rl_repo")

import numpy as np
import ml_dtypes

import concourse.bass as bass
import concourse.tile as tile
from concourse import bacc
from concourse import mybir
from concourse import bass_utils

B, S, D, H, DH = 4, 2048, 1024, 16, 64
HL = 8              # heads per core
NCORES = 8
F32 = mybir.dt.float32
BF16 = mybir.dt.bfloat16
EXP = mybir.ActivationFunctionType.Exp

NDC = D // 128      # 8 d-chunks of 128
NKC = S // 128      # 16 s_k chunks of 128
NSL = 4             # x column slices of 512

DEBUG_DUMPS = False  # adds intermediate DRAM outputs for numeric bisection


def _kernel_body(tc):
    nc = tc.nc
    xT = nc.dram_tensor("xT", (D, S), BF16, kind="ExternalInput").ap()
    wq = nc.dram_tensor("wq", (D, HL * DH), BF16, kind="ExternalInput").ap()
    wk = nc.dram_tensor("wk", (D, HL * DH), BF16, kind="ExternalInput").ap()
    wv = nc.dram_tensor("wv", (D, HL * DH), BF16, kind="ExternalInput").ap()
    wo = nc.dram_tensor("wo", (HL * DH, D), BF16, kind="ExternalInput").ap()
    out = nc.dram_tensor("out", (S, D), F32, kind="ExternalOutput").ap()

    # load queues: NEVER the scalar queue -- HWDGE DMA triggers hold the
    # issuing engine's SEQ ~625ns each, and ACT's SEQ must stay free for exp
    qs = [nc.sync, nc.gpsimd]

    with tc.tile_pool(name="persist", bufs=1) as persist, \
         tc.tile_pool(name="ptp", bufs=32) as ptp, \
         tc.tile_pool(name="stgp", bufs=20) as stgp, \
         tc.tile_pool(name="rrp", bufs=4) as rrp, \
         tc.tile_pool(name="stp", bufs=5) as stp, \
         tc.tile_pool(name="psS", bufs=2, space="PSUM") as psS, \
         tc.tile_pool(name="psO", bufs=2, space="PSUM") as psO, \
         tc.tile_pool(name="psG", bufs=2, space="PSUM") as psG:

        # ---------------- persistent tiles ----------------
        xt = [persist.tile([128, S], BF16, name=f"xt{dc}", tag=f"xt{dc}")
              for dc in range(NDC)]
        wqt = [persist.tile([128, HL * DH], BF16, name=f"wq{dc}", tag=f"wq{dc}")
               for dc in range(NDC)]
        wkt = [persist.tile([128, HL * DH], BF16, name=f"wk{dc}", tag=f"wk{dc}")
               for dc in range(NDC)]
        wvt = [persist.tile([128, HL * DH], BF16, name=f"wv{dc}", tag=f"wv{dc}")
               for dc in range(NDC)]
        wo_t = [persist.tile([128, D], BF16, name=f"wo{p}", tag=f"wo{p}")
                for p in range(4)]
        qt = [persist.tile([128, S], BF16, name=f"qt{p}", tag=f"qt{p}") for p in range(4)]
        kt = [persist.tile([128, S], BF16, name=f"kt{p}", tag=f"kt{p}") for p in range(4)]
        vv = [persist.tile([128, HL * 65], BF16, name=f"v{sc}", tag=f"v{sc}")
              for sc in range(NKC)]
        otpair = {q2: [persist.tile([128, 1024], BF16, name=f"otp_{q2}_{p}",
                                    tag=f"otp_{q2}_{p}") for p in range(4)]
                  for q2 in range(2)}

        # -------- input DMAs: xc0, wq, xc1, wk, xc2, xc3, wv, wo ---------
        qi = [0]

        # 5:3 sync/gpsimd weighting -- sync's HWDGE trigger (625ns) outpaces
        # gpsimd's software-DGE (~1038ns on Pool), so balance completion
        LOADPAT = [0, 1, 0, 0, 1, 0, 0, 1]

        def load(dst, src):
            qs[LOADPAT[qi[0] % 8]].dma_start(out=dst, in_=src)
            qi[0] += 1

        def load_w(dst_list, src):
            for dc in range(NDC):
                load(dst_list[dc], src[dc * 128:(dc + 1) * 128, :])

        def load_xcol(sl, width=512):
            for dc in range(NDC):
                load(xt[dc][:, sl * 512:sl * 512 + width],
                     xT[dc * 128:(dc + 1) * 128, sl * 512:sl * 512 + width])

        # x cols 0:1024 as small early DMAs (prologue + early-score gates);
        # cols 1024:2048 as one batched DMA per d-chunk (fewer triggers).
        # Interleave xc0/wq per d-chunk so the first qt accumulation chain
        # paces with DMA arrival instead of waiting for the full slices.
        for dc in range(NDC):
            load(xt[dc][:, 0:512], xT[dc * 128:(dc + 1) * 128, 0:512])
            load(wqt[dc], wq[dc * 128:(dc + 1) * 128, :])
        for dc in range(NDC):
            load(wkt[dc], wk[dc * 128:(dc + 1) * 128, :])
            load(xt[dc][:, 512:1024], xT[dc * 128:(dc + 1) * 128, 512:1024])
        load_xcol(2, width=1024)
        load_w(wvt, wv)
        for p in range(4):
            load(wo_t[p], wo[p * 128:(p + 1) * 128, :])
        # explicit zero bias for exp: the implicit const AP would be DMA'd
        # behind all input loads on the shared DMA device, stalling exp #0
        zbias = persist.tile([128, 1], F32, name="zbias", tag="zbias")
        nc.vector.memset(zbias, 0.0)
        for sc in range(NKC):
            nc.vector.memset(vv[sc], 1.0)



        # ---------------- PE work-item generators (yield = step cycles) ---
        def proj_chain(kind, p, sl):
            """qt/kt chain: 8 accumulating matmuls -> copy to qt/kt bf16.
            Finish ops bundled with the LAST matmul (before final yield)."""
            w = wqt if kind == "q" else wkt
            dstl = qt if kind == "q" else kt
            ps = psG.tile([128, 512], F32, name=f"pg_{kind}{p}_{sl}", tag="pg")
            for dc in range(NDC):
                nc.tensor.matmul(ps, w[dc][:, p * 128:(p + 1) * 128],
                                 xt[dc][:, sl * 512:(sl + 1) * 512],
                                 start=(dc == 0), stop=(dc == NDC - 1))
                if dc == NDC - 1:
                    nc.vector.tensor_copy(dstl[p][:, sl * 512:(sl + 1) * 512], ps)
                yield 512

        def v_chain(hg, nh, sc):
            """V chains for head-group hg of width nh heads, sk-chunk sc.
            psum (128, nh*64) accumulated over dc, scattered into vv[sc]."""
            w0 = hg * 64
            ps = psG.tile([128, 512], F32, name=f"pgv_{hg}_{sc}", tag="pg")
            pv = ps[:, 0:nh * 64]
            for dc in range(NDC):
                nc.tensor.matmul(pv, xt[dc][:, sc * 128:(sc + 1) * 128],
                                 wvt[dc][:, w0:w0 + nh * 64],
                                 start=(dc == 0), stop=(dc == NDC - 1))
                if dc == NDC - 1:
                    vsrc = pv.rearrange("p (h x) -> p h x", x=64)
                    vdst = vv[sc].rearrange("p (h x) -> p h x", x=65)
                    nc.vector.tensor_copy(vdst[:, hg:hg + nh, 0:64], vsrc)
                if dc % 2 == 1:
                    yield nh * 128

        def outproj_chain(q2, j, dcol):
            """pf (128 sq, 512) = sum_p otpair[p][:, j]^T @ wo_t[p][:, dcol].
            q2=1 chains run in the tail where the scores ring is idle: give
            every other chain a psS slot so 4 PSUM bufs rotate, hiding the
            staging-copy turnaround."""
            if q2 == 1 and (j + dcol) % 2 == 0:
                pf = psS.tile([128, 1024], F32, name=f"pf_{q2}_{j}_{dcol}",
                              tag="ps")[:, 0:512]
            else:
                pf = psG.tile([128, 512], F32, name=f"pf_{q2}_{j}_{dcol}", tag="pg")
            for p in range(4):
                nc.tensor.matmul(pf, otpair[q2][p][:, j * 128:(j + 1) * 128],
                                 wo_t[p][:, dcol * 512:(dcol + 1) * 512],
                                 start=(p == 0), stop=(p == 3))
                if p == 3:
                    st = stp.tile([128, 512], F32, name=f"st_{q2}_{j}_{dcol}", tag="st")
                    if q2 == 1:
                        # tail: exps are done, ACT is idle -- keep DVE free
                        nc.scalar.copy(st, pf)
                    else:
                        nc.vector.tensor_copy(st, pf)
                    # sync queue: gpsimd's software-DGE trigger costs ~1us of
                    # Pool time per store and serializes the out-proj tail
                    nc.sync.dma_start(
                        out=out[q2 * 1024 + j * 128: q2 * 1024 + (j + 1) * 128,
                                dcol * 512:(dcol + 1) * 512],
                        in_=st)
                yield 512

        filler_q = []
        op_last = []    # q2 values whose j=7 out-proj is pending release

        def filler(budget):
            while op_last:
                q2l = op_last.pop(0)
                for dcol in range(2):
                    filler_q.append(outproj_chain(q2l, 7, dcol))
            while budget > 0 and filler_q:
                try:
                    budget -= next(filler_q[0])
                except StopIteration:
                    filler_q.pop(0)

        # ---------------- attention block machinery ----------------
        pts_map = {}
        stage_map = {}

        def attn_chain(blk, j):
            """Flipped attnV for sq-tile j of block blk, plus normalize,
            transpose, and (for the completing pair) out-proj scheduling."""
            h, q2 = blk
            p, rh = h // 2, h % 2
            pts = pts_map[blk]
            po = psO.tile([128, 65], F32, name=f"po_{h}_{q2}_{j}", tag="po")
            for kc in range(NKC):
                nc.tensor.matmul(po, pts[kc][:, j * 128:(j + 1) * 128],
                                 vv[kc][:, h * 65:(h + 1) * 65],
                                 start=(kc == 0), stop=(kc == NKC - 1))
            rr = rrp.tile([128, 1], F32, name=f"rr_{h}_{q2}_{j}", tag="rr")
            nc.vector.reciprocal(rr, po[:, 64:65])
            if rh == 0:
                stg = stgp.tile([128, 128], BF16, name=f"sg_{q2}_{p}_{j}", tag="sg")
                stage_map[(q2, p, j)] = stg
            else:
                stg = stage_map[(q2, p, j)]
            nc.vector.tensor_scalar_mul(stg[:, rh * 64:(rh + 1) * 64],
                                        po[:, 0:64], rr)
            if rh == 1:
                nc.sync.dma_start_transpose(
                    otpair[q2][p][:, j * 128:(j + 1) * 128], stg)
                if p == 3:
                    # release the out-proj chains of the PREVIOUS j now (its
                    # transpose DMA has settled), never the just-emitted one
                    # (head-of-line stall on the PE stream)
                    if j > 0:
                        for dcol in range(2):
                            filler_q.append(outproj_chain(q2, j - 1, dcol))
                    if j == 7:
                        op_last.append(q2)

        blk_prev = [None]

        def scores_block(blk, prev_start_kc=1, frate=1024):
            """Score+exp tiles for blk, interleaving prev block's attnV
            chains (j = (kc - prev_start_kc)//2 at odd offsets) + filler."""
            h, q2 = blk
            p, rh = h // 2, h % 2
            rsl = slice(rh * 64, (rh + 1) * 64)
            prev = blk_prev[0]
            pts = []
            pts_map[blk] = pts
            jdone = 0
            for kc in range(NKC):
                ps = psS.tile([128, 1024], F32, name=f"ps_{h}_{q2}_{kc}", tag="ps")
                for half in range(2):
                    nc.tensor.matmul(
                        ps[:, half * 512:(half + 1) * 512],
                        kt[p][rsl, kc * 128:(kc + 1) * 128],
                        qt[p][rsl, q2 * 1024 + half * 512: q2 * 1024 + (half + 1) * 512],
                        start=True, stop=True)
                pe = ptp.tile([128, 1024], BF16, name=f"pt_{h}_{q2}_{kc}", tag="pt")
                nc.scalar.activation(pe, ps, EXP, bias=zbias, scale=0.125)
                pts.append(pe)
                if (prev is not None and kc >= prev_start_kc
                        and (kc - prev_start_kc) % 2 == 0 and jdone < 8):
                    attn_chain(prev, jdone)
                    jdone += 1
                filler(frate)
            if prev is not None:
                while jdone < 8:
                    attn_chain(prev, jdone)
                    jdone += 1
                    filler(frate)
            blk_prev[0] = blk

        # ---------------- prologue: pair-0 qt/kt for first scores --------
        # direct: q-sl0, k-sl0, q-sl1 (kc0 scores need qt sl0+sl1, kt sl0);
        # k-sl1..3 + q-sl2,3 at the filler head (consumed before their kc).
        for chain in (proj_chain("q", 0, 0), proj_chain("k", 0, 0),
                      proj_chain("q", 0, 1)):
            for _ in chain:
                pass

        # filler supply in deadline order (see block order below):
        for sl in (1, 2, 3):
            filler_q.append(proj_chain("k", 0, sl))
        filler_q.append(proj_chain("q", 0, 2))
        filler_q.append(proj_chain("q", 0, 3))
        for sc in range(NKC):
            filler_q.append(v_chain(0, 1, sc))      # head 0 V (early deadline)
        for sc in range(NKC):
            filler_q.append(v_chain(1, 1, sc))      # head 1 V
        for p in range(1, 4):
            for sl in range(NSL):
                filler_q.append(proj_chain("k", p, sl))
            for sl in range(NSL):
                filler_q.append(proj_chain("q", p, sl))
            for sc in range(NKC):
                filler_q.append(v_chain(2 * p, 2, sc))  # pair p V

        # ---- main loop: pairs 0-2 pair-major (spreads projection/V filler
        # deadlines); pair 3 q2-major so attnV((7,0))'s transposes release
        # out-proj q2=0 two blocks before the end.
        blocks = [(0, 0), (0, 1), (1, 0), (1, 1), (2, 0), (2, 1), (3, 0), (3, 1),
                  (4, 0), (4, 1), (5, 0), (5, 1), (6, 0), (7, 0), (6, 1), (7, 1)]
        for i, blk in enumerate(blocks):
            # block0: no prev attnV -> spend the slack on filler.
            # block1: defer block0's attnV until head-0 V filler completes.
            scores_block(blk, prev_start_kc=7 if i == 1 else 1,
                         frate=1536 if i < 2 else 1024)

        # tail: attnV of the last block (all chains first -- their pair-3
        # transposes append the final out-proj chains), then drain.
        last = blk_prev[0]
        for j in range(8):
            attn_chain(last, j)
        while filler_q:
            filler(65536)

        if DEBUG_DUMPS:
            dbg = {
                "d_qt0": qt[0], "d_kt0": kt[0], "d_vv0": vv[0],
                "d_ot00": otpair[0][0], "d_ot03": otpair[0][3],
            }
            for nm, t_ in dbg.items():
                shp = (t_.shape[0], t_.shape[1])
                dt_ = nc.dram_tensor(nm, shp, t_.dtype, kind="ExternalOutput").ap()
                nc.sync.dma_start(out=dt_, in_=t_[:, :])


_NC_CACHE = None


def _get_nc():
    global _NC_CACHE
    if _NC_CACHE is None:
        nc = bacc.Bacc("TRN2", target_bir_lowering=False, debug=False)
        with tile.TileContext(nc) as tc:
            _kernel_body(tc)
        nc.compile()
        _NC_CACHE = nc
    return _NC_CACHE


def _shard_inputs(x, Wq, Wk, Wv, Wo):
    bf = ml_dtypes.bfloat16
    in_maps = []
    for c in range(NCORES):
        b, g = c // 2, c % 2
        xT = np.ascontiguousarray(x[b].T).astype(bf)
        sl = slice(HL * g, HL * (g + 1))
        wq_s = np.ascontiguousarray(Wq[sl].transpose(1, 0, 2).reshape(D, HL * DH)).astype(bf)
        wk_s = np.ascontiguousarray(Wk[sl].transpose(1, 0, 2).reshape(D, HL * DH)).astype(bf)
        wv_s = np.ascontiguousarray(Wv[sl].transpose(1, 0, 2).reshape(D, HL * DH)).astype(bf)
        wo_s = np.ascontiguousarray(Wo[HL * DH * g: HL * DH * (g + 1), :]).astype(bf)
        in_maps.append({"xT": xT, "wq": wq_s, "wk": wk_s, "wv": wv_s, "wo": wo_s})
    return in_maps


def kernel(**inputs):
    x = np.asarray(inputs["x"], dtype=np.float32)
    Wq = np.asarray(inputs["Wq"], dtype=np.float32)
    Wk = np.asarray(inputs["Wk"], dtype=np.float32)
    Wv = np.asarray(inputs["Wv"], dtype=np.float32)
    Wo = np.asarray(inputs["Wo"], dtype=np.float32)

    nc = _get_nc()
    in_maps = _shard_inputs(x, Wq, Wk, Wv, Wo)
    res = None
    for attempt in range(3):
        try:
            res = bass_utils.run_bass_kernel_spmd(nc, in_maps, core_ids=list(range(NCORES)))
            break
        except Exception:
            # transient axon/NRT device errors recover on retry
            if attempt == 2:
                raise
            import time
            time.sleep(20)
    outs = [res.results[c]["out"] for c in range(NCORES)]
    full = np.stack([outs[2 * b] + outs[2 * b + 1] for b in range(B)], axis=0)
    return full.astype(np.float32)
